# revision 19
# baseline (speedup 1.0000x reference)
"""Trainium2 Bass kernel for nn_ASSC_66657892434080.

Reference computation (per batch sample b, data-parallel over 8 cores):
    q = wq @ x_1[b] + bq ; k = wk @ x[b] + bk          (1x1 convs)
    proj_query = PSP(q) [256,280] ; proj_key = PSP(k) [32,280]
    aff = sigmoid(proj_query @ proj_key^T)             [256,32]
    agg_w = (aff @ con.reshape(32, 256*9))             per-sample 3x3 weights
    out[b] = conv3x3(x_1[b], agg_w)                    (grouped conv, groups=B)

Key restructuring (validated vs reference in numpy to ~1e-6 rel):
  * PSP (adaptive-avg-pool pyramid) is linear, so PSP(wq@x+bq) = wq@PSP(x)+bq.
    The full-res q/k tensors are never materialized.
  * PSP itself = cumulative-sum along W, bin-differencing, cumsum along H,
    bin-differencing -> all on the Vector engine.
  * The grouped 3x3 conv = 9 shifted matmuls accumulating in PSUM over
    (tap, cin-chunk); contraction over cin=128/chunk on the partition dim.
  * con is host-reordered to [32, tap*256+cin] so the synthesized weights
    come out directly as the matmul lhsT tiles [cin, cout] per tap.
"""

import numpy as np
import concourse.bass as bass
import concourse.bacc as bacc
import concourse.tile as tile
import concourse.mybir as mybir
import bass_rust
from concourse.bass_utils import run_bass_kernel_spmd

B, C, H, W = 8, 256, 96, 96
C8 = 32
HW = H * W                      # 9216
POOL_SIZES = (1, 3, 5, 7, 14)   # -> 30 1-D bins, 280 2-D positions
NB = sum(POOL_SIZES)            # 30
NP = sum(s * s for s in POOL_SIZES)  # 280
STRIP = 32                      # pooling strip rows
NSTRIP = H // STRIP             # 3
ROWS_PER_SCHUNK = 4             # conv output rows per PSUM chunk
NSCHUNK = H // ROWS_PER_SCHUNK  # 24
SCHUNK = ROWS_PER_SCHUNK * W    # 384
F32 = mybir.dt.float32
F32R = mybir.dt.float32r
BF16 = mybir.dt.bfloat16


def _pool_bins(n, s):
    return [((i * n) // s, -((-(i + 1) * n) // s)) for i in range(s)]


WBINS = [b for s in POOL_SIZES for b in _pool_bins(W, s)]   # 30 (ws, we)
HBINS = {s: _pool_bins(H, s) for s in POOL_SIZES}
JBASE = {}
B280 = {}
_j = _p = 0
for _s in POOL_SIZES:
    JBASE[_s] = _j
    B280[_s] = _p
    _j += _s
    _p += _s * _s


def _pool_indicator():
    """Mk [9216, 280] bf16: 1.0 where spatial (h,w) falls in pooled bin p."""
    import ml_dtypes
    Mk = np.zeros((H * W, NP), np.float32)
    for s in POOL_SIZES:
        hb, wb = _pool_bins(H, s), _pool_bins(W, s)
        for o, (hs, he) in enumerate(hb):
            for p, (ws, we) in enumerate(wb):
                col = B280[s] + o * s + p
                for h in range(hs, he):
                    Mk[h * W + ws:h * W + we, col] = 1.0
    return Mk.astype(ml_dtypes.bfloat16)


def _area_inv():
    ai = np.zeros(NP, np.float32)
    for s in POOL_SIZES:
        hb, wb = _pool_bins(H, s), _pool_bins(W, s)
        for o, (hs, he) in enumerate(hb):
            for p, (ws, we) in enumerate(wb):
                ai[B280[s] + o * s + p] = 1.0 / ((he - hs) * (we - ws))
    return ai


def _split_multiwait_ctrl(nc, default_limit=1):
    """walrus in this container rejects instructions carrying more than one
    sem wait; move extras onto preceding same-engine drains (the engine
    blocks on those first, preserving semantics).  NEVER split PE
    instructions: the PE queue is a reorder window that pulls LDWEIGHTS
    ahead of in-flight work, so a wait moved onto a separate drain no
    longer gates the next matmul's weight load (observed as stale-weight
    garbage on HW).  Matmult accepts multiple waits in this walrus."""
    for f in nc.m.functions:
        for bb in f.blocks:
            new_list = []
            for inst in bb.instructions:
                si = inst.sync_info
                waits = list(si.on_wait) if si and si.on_wait else []
                mw = default_limit
                if getattr(inst, "engine", None) == mybir.EngineType.PE:
                    mw = 99
                if len(waits) > mw:
                    for k, w in enumerate(waits[:-mw]):
                        pre = mybir.InstDrain(name=f"{inst.name}-w{k}", ins=[], outs=[])
                        pre.engine = inst.engine
                        pre.sync_info = bass_rust.SyncInfo(on_wait=[w], on_update=[])
                        new_list.append(pre)
                    inst.sync_info = bass_rust.SyncInfo(
                        on_wait=waits[-mw:],
                        on_update=list(si.on_update) if si.on_update else [],
                    )
                new_list.append(inst)
            bb.instructions[:] = new_list


def _emit_psp(nc, P_out, src2d, F1, G, F2):
    """Pool one 128-channel chunk into P_out [128, 280] (raw bin SUMS).

    src2d: callable strip -> (flat 2D AP covering rows [strip*32, +32),
    rowlen, col_off) where element (r, w) of the strip lives at flat index
    r*rowlen + col_off + w.  Extra (zero-pad) columns between rows are
    harmless: the cumsum carries through them and bin differences stay
    within-row.  G layout: [128, NB*H] flat j*96+h (j = 1-D W-bin index).
    """
    add, byp, = mybir.AluOpType.add, mybir.AluOpType.bypass
    for strip in range(NSTRIP):
        h0 = strip * STRIP
        src, rowlen, coff = src2d(strip)
        n = STRIP * rowlen
        nc.vector.memset(F1[:, 0:1], 0.0)
        nc.vector.tensor_tensor_scan(F1[:, 1:1 + n], src, src, 0.0, add, byp)
        # F1 col (r*rowlen + coff + w) = cumsum through (r, w-1)
        for j, (ws, we) in enumerate(WBINS):
            nc.vector.tensor_sub(
                G[:, j * H + h0: j * H + h0 + STRIP],
                F1[:, coff + we: coff + we + (STRIP - 1) * rowlen + 1: rowlen],
                F1[:, coff + ws: coff + ws + (STRIP - 1) * rowlen + 1: rowlen],
            )
    nc.vector.memset(F2[:, 0:1], 0.0)
    nc.vector.tensor_tensor_scan(F2[:, 1:1 + NB * H], G[:, :], G[:, :], 0.0, add, byp)
    for s in POOL_SIZES:
        jb = JBASE[s]
        for o, (hs, he) in enumerate(HBINS[s]):
            nc.vector.tensor_sub(
                P_out[:, B280[s] + o * s: B280[s] + o * s + s],
                F2[:, jb * H + he: jb * H + he + (s - 1) * H + 1: H],
                F2[:, jb * H + hs: jb * H + hs + (s - 1) * H + 1: H],
            )




def _fix_ldweights_waits(nc):
    """Tile legalization splits 2-byte matmuls into standalone InstLdweights +
    InstMatmult, but can leave the wait that gates the WEIGHT data on the
    matmul -- after the weights were already latched -> stale-weight races on
    HW.  Move every wait of the pair onto the ldweights (waiting earlier is
    always safe; LDW-LDW order is preserved by the PE queue).  walrus allows
    only one wait per ldweights, so extra waits become duplicated ldweights
    (reloading the same weights is idempotent)."""
    import copy
    for f in nc.m.functions:
        for bb in f.blocks:
            insts = bb.instructions
            new_list = []
            i = 0
            while i < len(insts):
                inst = insts[i]
                nxt = insts[i + 1] if i + 1 < len(insts) else None
                if (type(inst).__name__ == "InstLdweights" and nxt is not None
                        and type(nxt).__name__ == "InstMatmult"):
                    wl = list(inst.sync_info.on_wait) if inst.sync_info and inst.sync_info.on_wait else []
                    wm = list(nxt.sync_info.on_wait) if nxt.sync_info and nxt.sync_info.on_wait else []
                    waits = wl + wm
                    mm_upd = list(nxt.sync_info.on_update) if nxt.sync_info and nxt.sync_info.on_update else []
                    ld_upd = list(inst.sync_info.on_update) if inst.sync_info and inst.sync_info.on_update else []
                    if len(waits) > 1:
                        for k, w in enumerate(waits[:-1]):
                            pre = copy.deepcopy(inst)
                            pre.name = f"{inst.name}-ldw{k}"
                            pre.sync_info = bass_rust.SyncInfo(on_wait=[w], on_update=[])
                            new_list.append(pre)
                        inst.sync_info = bass_rust.SyncInfo(on_wait=[waits[-1]], on_update=ld_upd)
                        nxt.sync_info = bass_rust.SyncInfo(on_wait=[], on_update=mm_upd)
                    elif len(waits) == 1:
                        inst.sync_info = bass_rust.SyncInfo(on_wait=[waits[0]], on_update=ld_upd)
                        nxt.sync_info = bass_rust.SyncInfo(on_wait=[], on_update=mm_upd)
                    new_list.append(inst)
                    new_list.append(nxt)
                    i += 2
                    continue
                new_list.append(inst)
                i += 1
            bb.instructions[:] = new_list


def build_kernel(conv_dtype=BF16, split_ctrl=True, debug_taps=False):
    nc = bacc.Bacc("TRN2", target_bir_lowering=False, debug=False)

    x1 = nc.dram_tensor("x1", [2, 128, HW], conv_dtype, kind="ExternalInput")
    xx = nc.dram_tensor("xx", [2, 128, HW], BF16, kind="ExternalInput")
    wqT = nc.dram_tensor("wqT", [2, 128, C], F32, kind="ExternalInput")
    wkT = nc.dram_tensor("wkT", [2, 128, C8], F32, kind="ExternalInput")
    bqb = nc.dram_tensor("bqb", [128, C], F32, kind="ExternalInput")
    bkb = nc.dram_tensor("bkb", [128, C8], F32, kind="ExternalInput")
    conr = nc.dram_tensor("conr", [C8, 9 * C], conv_dtype, kind="ExternalInput")
    ainv = nc.dram_tensor("ainv", [128, NP], F32, kind="ExternalInput")
    mk = nc.dram_tensor("mk", [HW, NP], BF16, kind="ExternalInput")
    out = nc.dram_tensor("out", [2, 128, HW], F32, kind="ExternalOutput")
    dbg = None
    if debug_taps:
        dbg = {
            "dbg_Pq0": nc.dram_tensor("dbg_Pq0", [128, NP], F32, kind="ExternalOutput"),
            "dbg_Pk0": nc.dram_tensor("dbg_Pk0", [128, NP], F32, kind="ExternalOutput"),
            "dbg_affT": nc.dram_tensor("dbg_affT", [C8, C], F32, kind="ExternalOutput"),
            "dbg_wsb": nc.dram_tensor("dbg_wsb", [128, 18 * C], F32, kind="ExternalOutput"),
            "dbg_xpad": nc.dram_tensor("dbg_xpad", [128, (H + 2) * (W + 2)], F32, kind="ExternalOutput"),
        }

    with tile.TileContext(nc) as tc:
        with (
            tc.tile_pool(name="consts", bufs=1) as cpool,
            tc.tile_pool(name="xpool", bufs=1) as xpool,
            tc.tile_pool(name="scratch", bufs=1) as spool,
            tc.tile_pool(name="wstage", bufs=2) as wpool,
            tc.tile_pool(name="ostage", bufs=4) as opool,
        ):
            # ---- constants ----
            wq_t = [cpool.tile([128, C], F32, tag=f"wq{i}", name=f"wq{i}") for i in range(2)]
            wk_t = [cpool.tile([128, C8], F32, tag=f"wk{i}", name=f"wk{i}") for i in range(2)]
            bq_t = cpool.tile([128, C], F32, tag="bq", name="bq")
            bk_t = cpool.tile([128, C8], F32, tag="bk", name="bk")
            ai_t = cpool.tile([128, NP], F32, tag="ai", name="ai")
            for i in range(2):
                nc.sync.dma_start(wq_t[i][:], wqT.ap()[i])
                nc.sync.dma_start(wk_t[i][:], wkT.ap()[i])
            nc.sync.dma_start(bq_t[:], bqb.ap())
            nc.sync.dma_start(bk_t[:], bkb.ap())
            nc.sync.dma_start(ai_t[:], ainv.ap())

            # ---- x_1 into zero-padded [128, 98, 98] tiles ----
            xpad = [xpool.tile([128, H + 2, W + 2], conv_dtype, tag=f"xpad{i}", name=f"xpad{i}") for i in range(2)]
            for cc in range(2):
                nc.vector.memset(xpad[cc][:, 0:H + 2:H + 1, :], 0.0)       # rows 0, 97
                nc.vector.memset(xpad[cc][:, 1:H + 1, 0:W + 2:W + 1], 0.0)  # cols 0, 97
            x1v = [x1.ap()[cc].rearrange("p (h w) -> p h w", w=W) for cc in range(2)]
            for cc in range(2):
                for strip in range(NSTRIP):
                    h0 = strip * STRIP
                    nc.sync.dma_start(
                        xpad[cc][:, 1 + h0:1 + h0 + STRIP, 1:1 + W],
                        x1v[cc][:, h0:h0 + STRIP, :],
                    )

            # ---- PE pooling: DMA-transpose 128-wide spatial chunks, then
            # accumulate x^T @ Mk (indicator matrix) in PSUM ----
            NK = HW // 128  # 72
            mk_t = cpool.tile([128, NK, NP], BF16, tag="mk", name="mk")
            mkv = mk.ap().rearrange("(k p) n -> k p n", p=128)
            for k0 in range(0, NK, 8):
                nc.sync.dma_start(mk_t[:, k0:k0 + 8, :], mkv[k0:k0 + 8].rearrange("k p n -> p k n"))
            xxv2 = xx.ap().rearrange("a c s -> (a c) s")
            x1v2 = x1.ap().rearrange("a c s -> (a c) s")
            xT = {}
            xT["xx"] = spool.tile([128, NK, 2 * 128], BF16, tag="xxT", name="xxT")
            xT["x1"] = spool.tile([128, NK, 2 * 128], BF16, tag="x1T", name="x1T")
            Pq = [cpool.tile([128, NP], F32, tag=f"Pq{i}", name=f"Pq{i}") for i in range(2)]
            Pk = [cpool.tile([128, NP], F32, tag=f"Pk{i}", name=f"Pk{i}") for i in range(2)]
            with tc.tile_pool(name="ppool", bufs=1, space="PSUM") as ppp:
                psums = {("xx", 0): ppp.tile([128, NP], F32, tag="pxx0", name="pxx0"),
                         ("xx", 1): ppp.tile([128, NP], F32, tag="pxx1", name="pxx1"),
                         ("x1", 0): ppp.tile([128, NP], F32, tag="px10", name="px10"),
                         ("x1", 1): ppp.tile([128, NP], F32, tag="px11", name="px11")}
                for name, srcv in (("xx", xxv2), ("x1", x1v2)):
                    for k in range(NK):
                        nc.sync.dma_start_transpose(
                            xT[name][:, k, :], srcv[:, k * 128:(k + 1) * 128])
                    for k in range(NK):
                        for cc in range(2):
                            nc.tensor.matmul(
                                psums[(name, cc)][:],
                                xT[name][:, k, cc * 128:(cc + 1) * 128],
                                mk_t[:, k, :],
                                start=(k == 0), stop=(k == NK - 1))
                # raw sums -> averages (PSUM -> SBUF)
                for cc in range(2):
                    nc.vector.tensor_mul(Pq[cc][:], psums[("x1", cc)][:], ai_t[:])
                    nc.vector.tensor_mul(Pk[cc][:], psums[("xx", cc)][:], ai_t[:])

            # ---- projections / affinity / weight synthesis ----
            PCH = [(0, 128), (128, 128), (256, 24)]
            pqT = [cpool.tile([n, C], F32, tag=f"pqT{i}", name=f"pqT{i}") for i, (_, n) in enumerate(PCH)]
            pkT = [cpool.tile([n, C8], F32, tag=f"pkT{i}", name=f"pkT{i}") for i, (_, n) in enumerate(PCH)]
            affT = cpool.tile([C8, C], conv_dtype, tag="affT", name="affT")
            w_sb = cpool.tile([128, 18 * C], conv_dtype, tag="w_sb", name="w_sb")

            with tc.tile_pool(name="psmall", bufs=2, space="PSUM") as pps:
                for i, (p0, n) in enumerate(PCH):
                    ps = pps.tile([n, C], F32, tag="ps", name="ps")
                    for cc in range(2):
                        nc.tensor.matmul(ps[:], Pq[cc][:, p0:p0 + n], wq_t[cc][:],
                                         start=(cc == 0), stop=(cc == 1))
                    nc.vector.tensor_add(pqT[i][:], ps[:], bq_t[:n, :])
                for i, (p0, n) in enumerate(PCH):
                    ps2 = pps.tile([n, C8], F32, tag="ps2", name="ps2")
                    for cc in range(2):
                        nc.tensor.matmul(ps2[:], Pk[cc][:, p0:p0 + n], wk_t[cc][:],
                                         start=(cc == 0), stop=(cc == 1))
                    nc.vector.tensor_add(pkT[i][:], ps2[:], bk_t[:n, :])
                pa = pps.tile([C8, C], F32, tag="pa", name="pa")
                for i in range(3):
                    nc.tensor.matmul(pa[:], pkT[i][:], pqT[i][:],
                                     start=(i == 0), stop=(i == 2))
                nc.scalar.activation(affT[:], pa[:], mybir.ActivationFunctionType.Sigmoid)
                for wc in range(18):
                    ct = wpool.tile([C8, 128], conv_dtype, tag="conr", name="conr")
                    nc.sync.dma_start(ct[:], conr.ap()[:, wc * 128:(wc + 1) * 128])
                    pw = pps.tile([128, C], F32, tag="pw", name="pw")
                    nc.tensor.matmul(pw[:], ct[:], affT[:],
                                     start=True, stop=True)
                    nc.scalar.copy(w_sb[:, wc * C:(wc + 1) * C], pw[:])

            if dbg is not None:
                dbg_f32 = cpool.tile([128, 18 * C], F32, tag="dbgf", name="dbgf")
                nc.sync.dma_start(dbg["dbg_Pq0"].ap(), Pq[0][:])
                nc.sync.dma_start(dbg["dbg_Pk0"].ap(), Pk[0][:])
                nc.vector.tensor_copy(dbg_f32[:C8, :C], affT[:])
                nc.sync.dma_start(dbg["dbg_affT"].ap(), dbg_f32[:C8, :C])
                nc.vector.tensor_copy(dbg_f32[:], w_sb[:])
                nc.sync.dma_start(dbg["dbg_wsb"].ap(), dbg_f32[:])
                dbg_xp = cpool.tile([128, (H + 2) * (W + 2)], F32, tag="dbgx", name="dbgx")
                nc.vector.tensor_copy(dbg_xp[:], xpad[0][:].rearrange("p h w -> p (h w)"))
                nc.sync.dma_start(dbg["dbg_xpad"].ap(), dbg_xp[:])

            # ---- the 3x3 grouped conv: 9 taps x 2 cin-chunks accumulate ----
            TAPS = [(t, cinc) for t in range(9) for cinc in range(2)]
            GRP = 4
            with tc.tile_pool(name="pconv", bufs=2, space="PSUM") as ppc:
                for coutc in range(2):
                    for grp in range(NSCHUNK // GRP):
                        pts = [ppc.tile([128, SCHUNK], F32, tag=f"cv{i}", name=f"cv{i}") for i in range(GRP)]
                        for ti, (t, cinc) in enumerate(TAPS):
                            dy, dx = t // 3, t % 3
                            wsl = w_sb[:, (t * 2 + cinc) * C + coutc * 128:
                                       (t * 2 + cinc) * C + coutc * 128 + 128]
                            for i in range(GRP):
                                s = grp * GRP + i
                                rhs = xpad[cinc][:, ROWS_PER_SCHUNK * s + dy:
                                                 ROWS_PER_SCHUNK * s + dy + ROWS_PER_SCHUNK,
                                                 dx:dx + W]
                                nc.tensor.matmul(pts[i][:], wsl, rhs,
                                                 start=(ti == 0), stop=(ti == 17))
                        for i in range(GRP):
                            s = grp * GRP + i
                            ot = opool.tile([128, SCHUNK], F32, tag="ot", name="ot")
                            nc.scalar.copy(ot[:], pts[i][:])
                            nc.sync.dma_start(
                                out.ap()[coutc][:, s * SCHUNK:(s + 1) * SCHUNK], ot[:])

    if split_ctrl:
        nc.compile()
        _fix_ldweights_waits(nc)
    return nc


_NC_CACHE = {}


def _get_nc():
    if "nc" not in _NC_CACHE:
        _NC_CACHE["nc"] = build_kernel()
    return _NC_CACHE["nc"]


def _tf32_round(x):
    u = np.ascontiguousarray(x, np.float32).view(np.uint32)
    u = (u + 0x0FFF + ((u >> 13) & 1)) & np.uint32(0xFFFFE000)
    return u.view(np.float32)


def _conv_cast(x):
    import ml_dtypes
    return np.ascontiguousarray(x, np.float32).astype(ml_dtypes.bfloat16)


def kernel(x_1, x, wq, bq, wk, bk, con):
    x_1 = _conv_cast(x_1)
    con = _conv_cast(con)
    x = _conv_cast(x)
    wq = np.asarray(wq, np.float32)
    bq = np.asarray(bq, np.float32)
    wk = np.asarray(wk, np.float32)
    bk = np.asarray(bk, np.float32)

    wqT_h = np.ascontiguousarray(wq.T).reshape(2, 128, C)
    wkT_h = np.ascontiguousarray(wk.T).reshape(2, 128, C8)
    bqb_h = np.tile(bq, (128, 1))
    bkb_h = np.tile(bk, (128, 1))
    # conr[k, tap*C + cin] = con[k, cin, dy, dx], tap = dy*3+dx
    conr_h = np.ascontiguousarray(con.transpose(2, 3, 1, 0).reshape(9 * C, C8).T)
    ainv_h = np.tile(_area_inv(), (128, 1))
    mk_h = _pool_indicator()

    in_maps = []
    for b in range(B):
        in_maps.append({
            "x1": x_1[b].reshape(2, 128, HW),
            "xx": x[b].reshape(2, 128, HW),
            "wqT": wqT_h, "wkT": wkT_h, "bqb": bqb_h, "bkb": bkb_h,
            "conr": conr_h, "ainv": ainv_h, "mk": mk_h,
        })
    global _last_in_maps
    _last_in_maps = in_maps
    nc = _get_nc()
    res = run_bass_kernel_spmd(nc, in_maps, list(range(B)))
    return np.stack([res.results[b]["out"].reshape(C, H, W) for b in range(B)])


# revision 20
# speedup vs baseline: 1.2687x; 1.2687x over previous
"""Trainium2 Bass kernel for nn_ASSC_66657892434080.

Reference computation (per batch sample b, data-parallel over 8 cores):
    q = wq @ x_1[b] + bq ; k = wk @ x[b] + bk          (1x1 convs)
    proj_query = PSP(q) [256,280] ; proj_key = PSP(k) [32,280]
    aff = sigmoid(proj_query @ proj_key^T)             [256,32]
    agg_w = (aff @ con.reshape(32, 256*9))             per-sample 3x3 weights
    out[b] = conv3x3(x_1[b], agg_w)                    (grouped conv, groups=B)

Key restructuring (validated vs reference in numpy to ~1e-6 rel):
  * PSP (adaptive-avg-pool pyramid) is linear, so PSP(wq@x+bq) = wq@PSP(x)+bq.
    The full-res q/k tensors are never materialized.
  * PSP itself = cumulative-sum along W, bin-differencing, cumsum along H,
    bin-differencing -> all on the Vector engine.
  * The grouped 3x3 conv = 9 shifted matmuls accumulating in PSUM over
    (tap, cin-chunk); contraction over cin=128/chunk on the partition dim.
  * con is host-reordered to [32, tap*256+cin] so the synthesized weights
    come out directly as the matmul lhsT tiles [cin, cout] per tap.
"""

import numpy as np
import concourse.bass as bass
import concourse.bacc as bacc
import concourse.tile as tile
import concourse.mybir as mybir
import bass_rust
from concourse.bass_utils import run_bass_kernel_spmd

B, C, H, W = 8, 256, 96, 96
C8 = 32
HW = H * W                      # 9216
POOL_SIZES = (1, 3, 5, 7, 14)   # -> 30 1-D bins, 280 2-D positions
NB = sum(POOL_SIZES)            # 30
NP = sum(s * s for s in POOL_SIZES)  # 280
STRIP = 96                      # pooling strip rows (full chunk)
NSTRIP = H // STRIP             # 3
ROWS_PER_SCHUNK = 4             # conv output rows per PSUM chunk
NSCHUNK = H // ROWS_PER_SCHUNK  # 24
SCHUNK = ROWS_PER_SCHUNK * W    # 384
F32 = mybir.dt.float32
F32R = mybir.dt.float32r
BF16 = mybir.dt.bfloat16


def _pool_bins(n, s):
    return [((i * n) // s, -((-(i + 1) * n) // s)) for i in range(s)]


WBINS = [b for s in POOL_SIZES for b in _pool_bins(W, s)]   # 30 (ws, we)
HBINS = {s: _pool_bins(H, s) for s in POOL_SIZES}
JBASE = {}
B280 = {}
_j = _p = 0
for _s in POOL_SIZES:
    JBASE[_s] = _j
    B280[_s] = _p
    _j += _s
    _p += _s * _s


def _pool_indicator():
    """Mk [9216, 280] bf16: 1.0 where spatial (h,w) falls in pooled bin p."""
    import ml_dtypes
    Mk = np.zeros((H * W, NP), np.float32)
    for s in POOL_SIZES:
        hb, wb = _pool_bins(H, s), _pool_bins(W, s)
        for o, (hs, he) in enumerate(hb):
            for p, (ws, we) in enumerate(wb):
                col = B280[s] + o * s + p
                for h in range(hs, he):
                    Mk[h * W + ws:h * W + we, col] = 1.0
    return Mk.astype(ml_dtypes.bfloat16)


def _area_inv():
    ai = np.zeros(NP, np.float32)
    for s in POOL_SIZES:
        hb, wb = _pool_bins(H, s), _pool_bins(W, s)
        for o, (hs, he) in enumerate(hb):
            for p, (ws, we) in enumerate(wb):
                ai[B280[s] + o * s + p] = 1.0 / ((he - hs) * (we - ws))
    return ai


def _split_multiwait_ctrl(nc, default_limit=1):
    """walrus in this container rejects instructions carrying more than one
    sem wait; move extras onto preceding same-engine drains (the engine
    blocks on those first, preserving semantics).  NEVER split PE
    instructions: the PE queue is a reorder window that pulls LDWEIGHTS
    ahead of in-flight work, so a wait moved onto a separate drain no
    longer gates the next matmul's weight load (observed as stale-weight
    garbage on HW).  Matmult accepts multiple waits in this walrus."""
    for f in nc.m.functions:
        for bb in f.blocks:
            new_list = []
            for inst in bb.instructions:
                si = inst.sync_info
                waits = list(si.on_wait) if si and si.on_wait else []
                mw = default_limit
                if getattr(inst, "engine", None) == mybir.EngineType.PE:
                    mw = 99
                if len(waits) > mw:
                    for k, w in enumerate(waits[:-mw]):
                        pre = mybir.InstDrain(name=f"{inst.name}-w{k}", ins=[], outs=[])
                        pre.engine = inst.engine
                        pre.sync_info = bass_rust.SyncInfo(on_wait=[w], on_update=[])
                        new_list.append(pre)
                    inst.sync_info = bass_rust.SyncInfo(
                        on_wait=waits[-mw:],
                        on_update=list(si.on_update) if si.on_update else [],
                    )
                new_list.append(inst)
            bb.instructions[:] = new_list


def _emit_psp(nc, P_out, src2d, F1, G, F2):
    """Pool one 128-channel chunk into P_out [128, 280] (raw bin SUMS).

    src2d: callable strip -> (flat 2D AP covering rows [strip*32, +32),
    rowlen, col_off) where element (r, w) of the strip lives at flat index
    r*rowlen + col_off + w.  Extra (zero-pad) columns between rows are
    harmless: the cumsum carries through them and bin differences stay
    within-row.  G layout: [128, NB*H] flat j*96+h (j = 1-D W-bin index).
    """
    add, byp, = mybir.AluOpType.add, mybir.AluOpType.bypass
    for strip in range(NSTRIP):
        h0 = strip * STRIP
        src, rowlen, coff = src2d(strip)
        n = STRIP * rowlen
        nc.vector.memset(F1[:, 0:1], 0.0)
        nc.vector.tensor_tensor_scan(F1[:, 1:1 + n], src, src, 0.0, add, byp)
        # F1 col (r*rowlen + coff + w) = cumsum through (r, w-1)
        for j, (ws, we) in enumerate(WBINS):
            nc.vector.tensor_sub(
                G[:, j * H + h0: j * H + h0 + STRIP],
                F1[:, coff + we: coff + we + (STRIP - 1) * rowlen + 1: rowlen],
                F1[:, coff + ws: coff + ws + (STRIP - 1) * rowlen + 1: rowlen],
            )
    nc.vector.memset(F2[:, 0:1], 0.0)
    nc.vector.tensor_tensor_scan(F2[:, 1:1 + NB * H], G[:, :], G[:, :], 0.0, add, byp)
    for s in POOL_SIZES:
        jb = JBASE[s]
        for o, (hs, he) in enumerate(HBINS[s]):
            nc.vector.tensor_sub(
                P_out[:, B280[s] + o * s: B280[s] + o * s + s],
                F2[:, jb * H + he: jb * H + he + (s - 1) * H + 1: H],
                F2[:, jb * H + hs: jb * H + hs + (s - 1) * H + 1: H],
            )




def _fix_ldweights_waits(nc):
    """Tile legalization splits 2-byte matmuls into standalone InstLdweights +
    InstMatmult, but can leave the wait that gates the WEIGHT data on the
    matmul -- after the weights were already latched -> stale-weight races on
    HW.  Move every wait of the pair onto the ldweights (waiting earlier is
    always safe; LDW-LDW order is preserved by the PE queue).  walrus allows
    only one wait per ldweights, so extra waits become duplicated ldweights
    (reloading the same weights is idempotent)."""
    import copy
    for f in nc.m.functions:
        for bb in f.blocks:
            insts = bb.instructions
            new_list = []
            i = 0
            while i < len(insts):
                inst = insts[i]
                nxt = insts[i + 1] if i + 1 < len(insts) else None
                if (type(inst).__name__ == "InstLdweights" and nxt is not None
                        and type(nxt).__name__ == "InstMatmult"):
                    wl = list(inst.sync_info.on_wait) if inst.sync_info and inst.sync_info.on_wait else []
                    wm = list(nxt.sync_info.on_wait) if nxt.sync_info and nxt.sync_info.on_wait else []
                    waits = wl + wm
                    mm_upd = list(nxt.sync_info.on_update) if nxt.sync_info and nxt.sync_info.on_update else []
                    ld_upd = list(inst.sync_info.on_update) if inst.sync_info and inst.sync_info.on_update else []
                    if len(waits) > 1:
                        for k, w in enumerate(waits[:-1]):
                            pre = copy.deepcopy(inst)
                            pre.name = f"{inst.name}-ldw{k}"
                            pre.sync_info = bass_rust.SyncInfo(on_wait=[w], on_update=[])
                            new_list.append(pre)
                        inst.sync_info = bass_rust.SyncInfo(on_wait=[waits[-1]], on_update=ld_upd)
                        nxt.sync_info = bass_rust.SyncInfo(on_wait=[], on_update=mm_upd)
                    elif len(waits) == 1:
                        inst.sync_info = bass_rust.SyncInfo(on_wait=[waits[0]], on_update=ld_upd)
                        nxt.sync_info = bass_rust.SyncInfo(on_wait=[], on_update=mm_upd)
                    new_list.append(inst)
                    new_list.append(nxt)
                    i += 2
                    continue
                new_list.append(inst)
                i += 1
            bb.instructions[:] = new_list


def build_kernel(conv_dtype=BF16, split_ctrl=True, debug_taps=False):
    nc = bacc.Bacc("TRN2", target_bir_lowering=False, debug=False)

    x1 = nc.dram_tensor("x1", [2, 128, HW], conv_dtype, kind="ExternalInput")
    xx = nc.dram_tensor("xx", [2, 128, HW], BF16, kind="ExternalInput")
    wqT = nc.dram_tensor("wqT", [2, 128, C], F32, kind="ExternalInput")
    wkT = nc.dram_tensor("wkT", [2, 128, C8], F32, kind="ExternalInput")
    bqb = nc.dram_tensor("bqb", [128, C], F32, kind="ExternalInput")
    bkb = nc.dram_tensor("bkb", [128, C8], F32, kind="ExternalInput")
    conr = nc.dram_tensor("conr", [C8, 9 * C], conv_dtype, kind="ExternalInput")
    ainv = nc.dram_tensor("ainv", [128, NP], F32, kind="ExternalInput")
    out = nc.dram_tensor("out", [2, 128, HW], F32, kind="ExternalOutput")
    dbg = None
    if debug_taps:
        dbg = {
            "dbg_Pq0": nc.dram_tensor("dbg_Pq0", [128, NP], F32, kind="ExternalOutput"),
            "dbg_Pk0": nc.dram_tensor("dbg_Pk0", [128, NP], F32, kind="ExternalOutput"),
            "dbg_affT": nc.dram_tensor("dbg_affT", [C8, C], F32, kind="ExternalOutput"),
            "dbg_wsb": nc.dram_tensor("dbg_wsb", [128, 18 * C], F32, kind="ExternalOutput"),
            "dbg_xpad": nc.dram_tensor("dbg_xpad", [128, (H + 2) * (W + 2)], F32, kind="ExternalOutput"),
        }

    with tile.TileContext(nc) as tc:
        with (
            tc.tile_pool(name="consts", bufs=1) as cpool,
            tc.tile_pool(name="xpool", bufs=1) as xpool,
            tc.tile_pool(name="scratch", bufs=1) as spool,
            tc.tile_pool(name="wstage", bufs=2) as wpool,
            tc.tile_pool(name="ostage", bufs=4) as opool,
        ):
            # ---- constants ----
            wq_t = [cpool.tile([128, C], F32, tag=f"wq{i}", name=f"wq{i}") for i in range(2)]
            wk_t = [cpool.tile([128, C8], F32, tag=f"wk{i}", name=f"wk{i}") for i in range(2)]
            bq_t = cpool.tile([128, C], F32, tag="bq", name="bq")
            bk_t = cpool.tile([128, C8], F32, tag="bk", name="bk")
            ai_t = cpool.tile([128, NP], F32, tag="ai", name="ai")
            for i in range(2):
                nc.sync.dma_start(wq_t[i][:], wqT.ap()[i])
                nc.sync.dma_start(wk_t[i][:], wkT.ap()[i])
            nc.sync.dma_start(bq_t[:], bqb.ap())
            nc.sync.dma_start(bk_t[:], bkb.ap())
            nc.sync.dma_start(ai_t[:], ainv.ap())

            # ---- x_1 into zero-padded [128, 98, 98] tiles ----
            xpad = [xpool.tile([128, H + 2, W + 2], conv_dtype, tag=f"xpad{i}", name=f"xpad{i}") for i in range(2)]
            for cc in range(2):
                nc.vector.memset(xpad[cc][:, 0:H + 2:H + 1, :], 0.0)       # rows 0, 97
                nc.vector.memset(xpad[cc][:, 1:H + 1, 0:W + 2:W + 1], 0.0)  # cols 0, 97
            x1v = [x1.ap()[cc].rearrange("p (h w) -> p h w", w=W) for cc in range(2)]
            for cc in range(2):
                for strip in range(NSTRIP):
                    h0 = strip * STRIP
                    nc.sync.dma_start(
                        xpad[cc][:, 1 + h0:1 + h0 + STRIP, 1:1 + W],
                        x1v[cc][:, h0:h0 + STRIP, :],
                    )

            # ---- pooling scratch (DVE cumsum + bin differencing) ----
            xbuf = spool.tile([128, HW], BF16, tag="xbuf", name="xbuf")
            F1 = spool.tile([128, STRIP * (W + 2) + 1], F32, tag="F1", name="F1")
            G = spool.tile([128, NB * H], F32, tag="G", name="G")
            F2 = spool.tile([128, NB * H + 1], F32, tag="F2", name="F2")
            Pq = [cpool.tile([128, NP], F32, tag=f"Pq{i}", name=f"Pq{i}") for i in range(2)]
            Pk = [cpool.tile([128, NP], F32, tag=f"Pk{i}", name=f"Pk{i}") for i in range(2)]

            # pool x (k-side) chunk by chunk through xbuf
            for cc in range(2):
                def src_xx(strip, _cc=cc):
                    nc.sync.dma_start(xbuf[:], xx.ap()[_cc])
                    return xbuf[:], W, 0
                _emit_psp(nc, Pk[cc], src_xx, F1, G, F2)
            # pool x_1 (q-side) straight from the padded tiles: rows are a
            # contiguous [96, 98] window; pad zeros flow through the cumsum
            xpflat = [xpad[cc][:].rearrange("p h w -> p (h w)") for cc in range(2)]
            for cc in range(2):
                def src_x1(strip, _cc=cc):
                    return (xpflat[_cc][:, (W + 2):(1 + H) * (W + 2)], W + 2, 1)
                _emit_psp(nc, Pq[cc], src_x1, F1, G, F2)

            # raw sums -> averages
            for cc in range(2):
                nc.vector.tensor_mul(Pq[cc][:], Pq[cc][:], ai_t[:])
                nc.vector.tensor_mul(Pk[cc][:], Pk[cc][:], ai_t[:])

            # ---- projections / affinity / weight synthesis ----
            PCH = [(0, 128), (128, 128), (256, 24)]
            pqT = [cpool.tile([n, C], F32, tag=f"pqT{i}", name=f"pqT{i}") for i, (_, n) in enumerate(PCH)]
            pkT = [cpool.tile([n, C8], F32, tag=f"pkT{i}", name=f"pkT{i}") for i, (_, n) in enumerate(PCH)]
            affT = cpool.tile([C8, C], conv_dtype, tag="affT", name="affT")
            w_sb = cpool.tile([128, 18 * C], conv_dtype, tag="w_sb", name="w_sb")

            with tc.tile_pool(name="psmall", bufs=2, space="PSUM") as pps:
                for i, (p0, n) in enumerate(PCH):
                    ps = pps.tile([n, C], F32, tag="ps", name="ps")
                    for cc in range(2):
                        nc.tensor.matmul(ps[:], Pq[cc][:, p0:p0 + n], wq_t[cc][:],
                                         start=(cc == 0), stop=(cc == 1))
                    nc.vector.tensor_add(pqT[i][:], ps[:], bq_t[:n, :])
                for i, (p0, n) in enumerate(PCH):
                    ps2 = pps.tile([n, C8], F32, tag="ps2", name="ps2")
                    for cc in range(2):
                        nc.tensor.matmul(ps2[:], Pk[cc][:, p0:p0 + n], wk_t[cc][:],
                                         start=(cc == 0), stop=(cc == 1))
                    nc.vector.tensor_add(pkT[i][:], ps2[:], bk_t[:n, :])
                pa = pps.tile([C8, C], F32, tag="pa", name="pa")
                for i in range(3):
                    nc.tensor.matmul(pa[:], pkT[i][:], pqT[i][:],
                                     start=(i == 0), stop=(i == 2))
                nc.scalar.activation(affT[:], pa[:], mybir.ActivationFunctionType.Sigmoid)
                for wc in range(18):
                    ct = wpool.tile([C8, 128], conv_dtype, tag="conr", name="conr")
                    nc.sync.dma_start(ct[:], conr.ap()[:, wc * 128:(wc + 1) * 128])
                    pw = pps.tile([128, C], F32, tag="pw", name="pw")
                    nc.tensor.matmul(pw[:], ct[:], affT[:],
                                     start=True, stop=True)
                    nc.scalar.copy(w_sb[:, wc * C:(wc + 1) * C], pw[:])

            if dbg is not None:
                dbg_f32 = cpool.tile([128, 18 * C], F32, tag="dbgf", name="dbgf")
                nc.sync.dma_start(dbg["dbg_Pq0"].ap(), Pq[0][:])
                nc.sync.dma_start(dbg["dbg_Pk0"].ap(), Pk[0][:])
                nc.vector.tensor_copy(dbg_f32[:C8, :C], affT[:])
                nc.sync.dma_start(dbg["dbg_affT"].ap(), dbg_f32[:C8, :C])
                nc.vector.tensor_copy(dbg_f32[:], w_sb[:])
                nc.sync.dma_start(dbg["dbg_wsb"].ap(), dbg_f32[:])
                dbg_xp = cpool.tile([128, (H + 2) * (W + 2)], F32, tag="dbgx", name="dbgx")
                nc.vector.tensor_copy(dbg_xp[:], xpad[0][:].rearrange("p h w -> p (h w)"))
                nc.sync.dma_start(dbg["dbg_xpad"].ap(), dbg_xp[:])

            # ---- the 3x3 grouped conv: 9 taps x 2 cin-chunks accumulate ----
            TAPS = [(t, cinc) for t in range(9) for cinc in range(2)]
            GRP = 4
            with tc.tile_pool(name="pconv", bufs=2, space="PSUM") as ppc:
                for coutc in range(2):
                    for grp in range(NSCHUNK // GRP):
                        pts = [ppc.tile([128, SCHUNK], F32, tag=f"cv{i}", name=f"cv{i}") for i in range(GRP)]
                        for ti, (t, cinc) in enumerate(TAPS):
                            dy, dx = t // 3, t % 3
                            wsl = w_sb[:, (t * 2 + cinc) * C + coutc * 128:
                                       (t * 2 + cinc) * C + coutc * 128 + 128]
                            for i in range(GRP):
                                s = grp * GRP + i
                                rhs = xpad[cinc][:, ROWS_PER_SCHUNK * s + dy:
                                                 ROWS_PER_SCHUNK * s + dy + ROWS_PER_SCHUNK,
                                                 dx:dx + W]
                                nc.tensor.matmul(pts[i][:], wsl, rhs,
                                                 start=(ti == 0), stop=(ti == 17))
                        for i in range(GRP):
                            s = grp * GRP + i
                            ot = opool.tile([128, SCHUNK], F32, tag="ot", name="ot")
                            nc.scalar.copy(ot[:], pts[i][:])
                            nc.sync.dma_start(
                                out.ap()[coutc][:, s * SCHUNK:(s + 1) * SCHUNK], ot[:])

    if split_ctrl:
        nc.compile()
        _fix_ldweights_waits(nc)
    return nc


_NC_CACHE = {}


def _get_nc():
    if "nc" not in _NC_CACHE:
        _NC_CACHE["nc"] = build_kernel()
    return _NC_CACHE["nc"]


def _tf32_round(x):
    u = np.ascontiguousarray(x, np.float32).view(np.uint32)
    u = (u + 0x0FFF + ((u >> 13) & 1)) & np.uint32(0xFFFFE000)
    return u.view(np.float32)


def _conv_cast(x):
    import ml_dtypes
    return np.ascontiguousarray(x, np.float32).astype(ml_dtypes.bfloat16)


def kernel(x_1, x, wq, bq, wk, bk, con):
    x_1 = _conv_cast(x_1)
    con = _conv_cast(con)
    x = _conv_cast(x)
    wq = np.asarray(wq, np.float32)
    bq = np.asarray(bq, np.float32)
    wk = np.asarray(wk, np.float32)
    bk = np.asarray(bk, np.float32)

    wqT_h = np.ascontiguousarray(wq.T).reshape(2, 128, C)
    wkT_h = np.ascontiguousarray(wk.T).reshape(2, 128, C8)
    bqb_h = np.tile(bq, (128, 1))
    bkb_h = np.tile(bk, (128, 1))
    # conr[k, tap*C + cin] = con[k, cin, dy, dx], tap = dy*3+dx
    conr_h = np.ascontiguousarray(con.transpose(2, 3, 1, 0).reshape(9 * C, C8).T)
    ainv_h = np.tile(_area_inv(), (128, 1))

    in_maps = []
    for b in range(B):
        in_maps.append({
            "x1": x_1[b].reshape(2, 128, HW),
            "xx": x[b].reshape(2, 128, HW),
            "wqT": wqT_h, "wkT": wkT_h, "bqb": bqb_h, "bkb": bkb_h,
            "conr": conr_h, "ainv": ainv_h,
        })
    global _last_in_maps
    _last_in_maps = in_maps
    nc = _get_nc()
    res = run_bass_kernel_spmd(nc, in_maps, list(range(B)))
    return np.stack([res.results[b]["out"].reshape(C, H, W) for b in range(B)])
